# revision 8
# baseline (speedup 1.0000x reference)
"""Multi-head causal attention (B=2, S=2048, D=1024, H=16) on 8 trn2 cores.

Sharding (Megatron TP over batch*heads): core c handles batch c//4 and the
4 heads 4*(c%4)..4*(c%4)+3.  Wq/Wk/Wv are column-sharded (each core gets the
256 rows of W* for its heads), Wo is row-sharded; the host sums the 4 partial
outputs per batch (the tensor-parallel all-reduce) and adds bo.

Device kernel (per core), matmuls in bf16 (f32r/f32 fallback via env):
  - qT/kT = W*_shard @ X.T   [256, 2048]  (head dim on partitions)
  - v     = X @ Wv_shard.T   [2048, 256]  (+ ones column per head for the
                                           softmax denominator)
  - per head: scoresT[s,t] = kT.T-tiles x qT  (causal tiles only),
    exp on ScalarE (scale=1/8), triangular-block mask on VectorE,
    outT[65, t] += v_ext.T @ attnT  (row 64 = denominator)
  - normalize per 512-token psum bank as soon as its accumulation stops:
    denom row -> DVE reciprocal -> PE broadcast (ones[1,64] x recip row)
    -> DVE multiply into outTn
  - out_partial[t, :] = outTn.T-tiles @ WoT_shard, emitted per bank as soon
    as both head pairs are normalized (hides the store tail)

Schedule: q/k projections first (DMA-overlapped); the hp0/chunk0 score+exp
pipeline starts immediately after (ScalarE is the attention bottleneck, so
it must start early), with the v projection woven between score steps and
the attn@V matmuls deferred until v lands.
"""

import os
import sys

sys.path.insert(0, "/opt/trn_rl_repo")

import ml_dtypes
import numpy as np

import concourse.bass as bass  # noqa: F401
import concourse.bass2jax as bass2jax
import concourse.tile as tile
from concourse import bacc, mybir
from concourse.bass_utils import run_bass_kernel_spmd

# Optional NEFF tee for local profiling (active only when the env var is set).
_orig_rename = bass2jax.rename_neff_tensors_and_patch_header


def _tee_rename(neff_path, mapping):
    data = _orig_rename(neff_path, mapping)
    tee = os.environ.get("BASS_MHA_NEFF_TEE")
    if tee:
        try:
            with open(tee, "wb") as f:
                f.write(data)
        except OSError:
            pass
    return data


bass2jax.rename_neff_tensors_and_patch_header = _tee_rename

F32 = mybir.dt.float32
AF = mybir.ActivationFunctionType

S = 2048  # sequence length
D = 1024  # model dim
HL = 256  # local head width (4 heads x 64)
DK = 64  # head dim
N_SI = S // 128  # 16 token tiles (partition dim of scoresT)

MODE = os.environ.get("BASS_MHA_DTYPE", "bf16")  # bf16 | f32r | f32
if MODE == "bf16":
    DT = mybir.dt.bfloat16
    NPDT = ml_dtypes.bfloat16
elif MODE == "f32r":
    DT = mybir.dt.float32r
    NPDT = np.float32
else:
    DT = F32
    NPDT = np.float32

LAST_EXEC_NS = None
_CACHED_NC = None


def _prep(a):
    """Cast a host array to the kernel's compute dtype (with fp32r rounding
    matching the compiler's fp32_to_fp32r when in f32r mode)."""
    a = np.ascontiguousarray(np.asarray(a, np.float32))
    if MODE == "bf16":
        return a.astype(ml_dtypes.bfloat16)
    if MODE == "f32r":
        bits = a.view(np.uint32).astype(np.uint64)
        rounded = (bits + 0x7FF + ((bits >> 12) & 1)) & 0xFFFFF000
        return rounded.astype(np.uint32).view(np.float32).reshape(a.shape)
    return a


def _build_kernel(tc):
    nc = tc.nc
    qt = nc.dram_tensor("qt", [D, S], DT, kind="ExternalInput").ap()
    kt = nc.dram_tensor("kt", [D, S], DT, kind="ExternalInput").ap()
    vt = nc.dram_tensor("vt", [D, S], DT, kind="ExternalInput").ap()
    wqt = nc.dram_tensor("wqt", [D, HL], DT, kind="ExternalInput").ap()
    wkt = nc.dram_tensor("wkt", [D, HL], DT, kind="ExternalInput").ap()
    wvt = nc.dram_tensor("wvt", [D, HL], DT, kind="ExternalInput").ap()
    wot = nc.dram_tensor("wot", [HL, D], DT, kind="ExternalInput").ap()
    mtri = nc.dram_tensor("mtri", [128, 128], DT, kind="ExternalInput").ap()
    vones = nc.dram_tensor("vones", [128, N_SI, 4, 1], DT, kind="ExternalInput").ap()
    out = nc.dram_tensor("out", [S, D], F32, kind="ExternalOutput").ap()

    consts = tc.alloc_tile_pool(name="consts", bufs=1)
    persist = tc.alloc_tile_pool(name="persist", bufs=1)
    xt_pool = tc.alloc_tile_pool(name="xt", bufs=16)
    attn_pool = tc.alloc_tile_pool(name="attn", bufs=16)
    nrm_pool = tc.alloc_tile_pool(name="nrm", bufs=4)
    out_pool = tc.alloc_tile_pool(name="outsb", bufs=3)
    sc_psum = tc.alloc_tile_pool(name="scps", bufs=2, space="PSUM")
    ot_psum = tc.alloc_tile_pool(name="otps", bufs=1, space="PSUM")

    # --- constants ---
    wq_sb = consts.tile([128, 8, HL], DT, name="wq_sb")
    wk_sb = consts.tile([128, 8, HL], DT, name="wk_sb")
    wv_sb = consts.tile([128, 8, HL], DT, name="wv_sb")
    wo_sb = consts.tile([128, 2, D], DT, name="wo_sb")
    mtri_sb = consts.tile([128, 128], DT, name="mtri_sb")
    ones_sb = consts.tile([1, 64], DT, name="ones_sb")
    nc.vector.memset(ones_sb, 1.0)
    nc.sync.dma_start(out=wq_sb, in_=wqt.rearrange("(n p) c -> p n c", p=128))

    # --- persistent activations ---
    qT = [persist.tile([128, S], DT, name=f"qT{i}", tag=f"qT{i}") for i in range(2)]
    kT = [persist.tile([128, S], DT, name=f"kT{i}", tag=f"kT{i}") for i in range(2)]
    # v with an appended ones column per head: [token_tile, si, head, 65]
    v_sb = persist.tile([128, N_SI, 4, DK + 1], DT, name="v_sb", tag="v_sb")
    outTn = [
        persist.tile([128, S], DT, name=f"outTn{i}", tag=f"outTn{i}") for i in range(2)
    ]

    # --- PE warm-up during the initial DMA wait: junk matmuls keep the HAM
    # activity window busy so the first projection matmuls run fast.
    junk = consts.tile([128, 512], DT, name="junk")
    nc.vector.memset(junk, 0.0)
    wps = sc_psum.tile([128, 512], F32, name="warm", tag="sc")
    for _ in range(8):
        nc.tensor.matmul(wps, junk[:, 0:128], junk, start=True, stop=True)

    # --- input tile DMAs --------------------------------------------------
    def load_xts(src, prefix):
        tiles = []
        for d in range(8):
            xtile = xt_pool.tile([128, S], DT, name=f"{prefix}_{d}", tag="xt")
            nc.sync.dma_start(out=xtile, in_=src[128 * d : 128 * d + 128, :])
            tiles.append(xtile)
        return tiles

    # --- q/k projections: psum[dq_tile, t] += wT[dtile, dq_tile].T @ xT[dtile, t]
    # One [128, 1024] psum group per (hp, token-half); while the ot banks hold
    # no accumulators ("quad" mode) rotate through all four psum slots so
    # groups never stall; once attn@V accumulators go live, rotate sc only.
    PSUM_TAGS = ["sc", "sc", "ot0", "ot1"]
    psum_rot = [0]
    psum_mode = ["quad"]

    def next_psum_tile(name):
        if psum_mode[0] == "quad":
            tag = PSUM_TAGS[psum_rot[0] % 4]
            psum_rot[0] += 1
        else:
            tag = "sc"
        pool = sc_psum if tag == "sc" else ot_psum
        return pool.tile([128, 1024], F32, name=name, tag=tag)

    def proj_group(xts, w_sb, hp, half, dst):
        ps = next_psum_tile(f"pj_{hp}_{half}")
        for d in range(8):
            for j in range(2):
                js = slice(512 * j, 512 * j + 512)
                ja = slice(1024 * half + 512 * j, 1024 * half + 512 * j + 512)
                nc.tensor.matmul(
                    ps[:, js],
                    w_sb[:, d, 128 * hp : 128 * hp + 128],
                    xts[d][:, ja],
                    start=(d == 0),
                    stop=(d == 7),
                )
        cols = slice(1024 * half, 1024 * half + 1024)
        nc.vector.tensor_copy(out=dst[:, cols], in_=ps)

    qxs = load_xts(qt, "xq")
    nc.sync.dma_start(out=wk_sb, in_=wkt.rearrange("(n p) c -> p n c", p=128))
    for hp in range(2):
        for half in range(2):
            proj_group(qxs, wq_sb, hp, half, qT[hp])
    kxs = load_xts(kt, "xk")
    nc.sync.dma_start(out=wv_sb, in_=wvt.rearrange("(n p) c -> p n c", p=128))
    nc.sync.dma_start(out=mtri_sb, in_=mtri)
    nc.sync.dma_start(out=v_sb[:, :, :, DK : DK + 1], in_=vones)
    for half in range(2):
        proj_group(kxs, wk_sb, 0, half, kT[0])
    vxs = load_xts(vt, "xv")
    nc.sync.dma_start(out=wo_sb, in_=wot.rearrange("(n p) c -> p n c", p=128))

    # --- attention helpers ------------------------------------------------
    def score_step(hp, h, si, ch):
        """Scores + exp + mask for head h (0..3), key tile si, chunk ch.
        Returns the bf16 attention-weights tile."""
        ch_lo = 1024 * ch
        t_min = 128 * si
        hr = 64 * (h % 2)
        banks = [tj for tj in (2 * ch, 2 * ch + 1) if 512 * tj + 512 > t_min]
        sc = next_psum_tile(f"sc_{h}_{si}_{ch}")
        for tj in banks:
            a = max(512 * tj, t_min)  # skip sub-diagonal columns
            rel = slice(a - ch_lo, 512 * tj - ch_lo + 512)
            nc.tensor.matmul(
                sc[:, rel],
                kT[hp][hr : hr + 64, t_min : t_min + 128],
                qT[hp][hr : hr + 64, a : 512 * tj + 512],
                start=True,
                stop=True,
            )
        att = attn_pool.tile([128, 1024], DT, name=f"at_{h}_{si}_{ch}", tag="at")
        off = max(t_min - ch_lo, 0)
        nc.scalar.activation(att[:, off:1024], sc[:, off:1024], AF.Exp, scale=0.125)
        if ch_lo <= t_min < ch_lo + 1024:
            nc.vector.tensor_mul(
                att[:, off : off + 128], att[:, off : off + 128], mtri_sb
            )
        return att

    def av_step(ot, h, si, ch, att):
        """outT[65, t] += v_ext.T @ attnT for head h, key tile si."""
        ch_lo = 1024 * ch
        t_min = 128 * si
        for tj in (2 * ch, 2 * ch + 1):
            if 512 * tj + 512 <= t_min:
                continue
            a = max(512 * tj, t_min)
            b = 512 * tj + 512
            nc.tensor.matmul(
                ot[0:65, a - ch_lo : b - ch_lo],
                v_sb[:, si, h, :],
                att[:, a - ch_lo : b - ch_lo],
                start=(si == 0),
                stop=(si == 4 * tj + 3),
                skip_group_check=True,
            )

    def norm_bank(hp, h, ch, tj, ot):
        """Normalize one completed 512-token psum bank of head h: outTn
        = outT[0:64] * broadcast(1/denom).  denom is outT row 64."""
        hr = 64 * (h % 2)
        rel = slice(512 * (tj - 2 * ch), 512 * (tj - 2 * ch) + 512)
        rrow = nrm_pool.tile([1, 512], DT, name=f"rr_{h}_{ch}_{tj}", tag="rrow")
        # bf16 reciprocal: ~0.2% rounding on the softmax scale, well inside
        # the tolerance; bf16 is required to feed the PE broadcast matmul.
        with nc.allow_low_precision(reason="softmax denom reciprocal to bf16"):
            nc.vector.reciprocal(rrow, ot[64:65, rel])
        bc = next_psum_tile(f"bc_{h}_{ch}_{tj}")
        nc.tensor.matmul(bc[0:64, 0:512], ones_sb, rrow, start=True, stop=True)
        bcs = nrm_pool.tile([64, 512], F32, name=f"bcs_{h}_{ch}_{tj}", tag="bcs")
        nc.vector.tensor_copy(out=bcs, in_=bc[0:64, 0:512])
        nc.vector.tensor_mul(
            outTn[hp][hr : hr + 64, 512 * tj : 512 * tj + 512],
            ot[0:64, rel],
            bcs,
        )

    def outproj_tile(tt):
        """out[t, :] = sum_k outTn[k, tt].T @ woT[k, :] for one token tile."""
        ts = slice(128 * tt, 128 * tt + 128)
        ps = next_psum_tile(f"op_{tt}")
        for kk in range(2):
            for nj in range(2):
                js = slice(512 * nj, 512 * nj + 512)
                nc.tensor.matmul(
                    ps[:, js],
                    outTn[kk][:, ts],
                    wo_sb[:, kk, js],
                    start=(kk == 0),
                    stop=(kk == 1),
                )
        osb = out_pool.tile([128, D], F32, name=f"osb_{tt}", tag="osb")
        nc.vector.tensor_copy(out=osb, in_=ps)
        nc.sync.dma_start(out=out[ts, :], in_=osb)

    def v_group(g):
        """v projection for key tiles 4g..4g+3 in one [128, 1024] psum group."""
        ps = next_psum_tile(f"vps_{g}")
        for k in range(4):
            si = 4 * g + k
            for d in range(8):
                nc.tensor.matmul(
                    ps[:, 256 * k : 256 * k + 256],
                    vxs[d][:, 128 * si : 128 * si + 128],
                    wv_sb[:, d, :],
                    start=(d == 0),
                    stop=(d == 7),
                )
        nc.vector.tensor_copy(
            out=v_sb[:, 4 * g : 4 * g + 4, :, 0:DK],
            in_=ps.rearrange("p (s h d) -> p s h d", s=4, h=4),
        )

    # --- Phase B: hp0/ch0 scores+exp immediately (ScalarE must start early);
    # the k-hp1 projection and the v projection are woven between score steps
    # (matching their DMA arrival), and attn@V is deferred until v lands.
    atts0 = {}
    for si in range(8):
        for h in (0, 1):
            atts0[(h, si)] = score_step(0, h, si, 0)
        if si in (1, 2):
            proj_group(kxs, wk_sb, 1, si - 1, kT[1])
        elif si >= 4:
            v_group(si - 4)
    psum_mode[0] = "duo"
    ot0 = {
        h: ot_psum.tile([128, 1024], F32, name=f"outT_{h}_0", tag=f"ot{h}")
        for h in (0, 1)
    }
    for si in range(8):
        for h in (0, 1):
            av_step(ot0[h], h, si, 0, atts0[(h, si)])
        if si == 3:
            for h in (0, 1):
                norm_bank(0, h, 0, 0, ot0[h])
    for h in (0, 1):
        norm_bank(0, h, 0, 1, ot0[h])

    # --- remaining chunks: zippered score/exp/AV pipeline per head pair.
    # hp1 interleaves the output projection for banks whose tokens are fully
    # normalized in both head pairs (hp0 ran first).
    def chunk(hp, ch, emit_outproj):
        heads = (2 * hp, 2 * hp + 1)
        ot = {
            h: ot_psum.tile([128, 1024], F32, name=f"outT_{h}_{ch}", tag=f"ot{h % 2}")
            for h in heads
        }
        si_max = 8 * ch + 7
        pend_op = []
        for si in range(si_max + 1):
            atts = [score_step(hp, h, si, ch) for h in heads]
            for h, att in zip(heads, atts):
                av_step(ot[h], h, si, ch, att)
            for tj in (2 * ch, 2 * ch + 1):
                if si == 4 * tj + 3:
                    for h in heads:
                        norm_bank(hp, h, ch, tj, ot[h])
                    if emit_outproj:
                        pend_op.extend(range(4 * tj, 4 * tj + 4))
            if pend_op and si % 2 == 1:
                outproj_tile(pend_op.pop(0))
        for tt in pend_op:
            outproj_tile(tt)

    chunk(0, 1, emit_outproj=False)
    chunk(1, 0, emit_outproj=True)
    chunk(1, 1, emit_outproj=True)

    for pool in (
        ot_psum,
        sc_psum,
        out_pool,
        nrm_pool,
        attn_pool,
        xt_pool,
        persist,
        consts,
    ):
        pool.release()


def _get_nc():
    global _CACHED_NC
    if _CACHED_NC is None:
        nc = bacc.Bacc("TRN2", target_bir_lowering=False, debug=False)
        with tile.TileContext(nc) as tc:
            _build_kernel(tc)
        nc.compile()
        _CACHED_NC = nc
    return _CACHED_NC


def kernel(Q, K, V, mask, Wq, Wk, Wv, Wo, bo):
    global LAST_EXEC_NS
    nc = _get_nc()
    mtri = np.triu(np.ones((128, 128), dtype=np.float32))
    in_maps = []
    for c in range(8):
        b, hg = c // 4, c % 4
        rs = slice(HL * hg, HL * hg + HL)
        in_maps.append(
            {
                "qt": _prep(np.asarray(Q, np.float32)[b].T),
                "kt": _prep(np.asarray(K, np.float32)[b].T),
                "vt": _prep(np.asarray(V, np.float32)[b].T),
                "wqt": _prep(np.asarray(Wq, np.float32)[rs].T),
                "wkt": _prep(np.asarray(Wk, np.float32)[rs].T),
                "wvt": _prep(np.asarray(Wv, np.float32)[rs].T),
                "wot": _prep(np.asarray(Wo, np.float32)[:, rs].T),
                "mtri": _prep(mtri),
                "vones": _prep(np.ones((128, N_SI, 4, 1), np.float32)),
            }
        )
    trace = os.environ.get("BASS_MHA_TRACE", "") == "1"
    res = run_bass_kernel_spmd(nc, in_maps, core_ids=list(range(8)), trace=trace)
    LAST_EXEC_NS = res.exec_time_ns
    outs = [res.results[c]["out"] for c in range(8)]
    bo = np.asarray(bo, np.float32)
    full = np.stack(
        [
            outs[0] + outs[1] + outs[2] + outs[3] + bo,
            outs[4] + outs[5] + outs[6] + outs[7] + bo,
        ]
    ).astype(np.float32)
    return full


# revision 14
# speedup vs baseline: 1.3783x; 1.3783x over previous
"""Multi-head causal attention (B=2, S=2048, D=1024, H=16) on 8 trn2 cores.

Sharding (Megatron TP over batch*heads): core c handles batch c//4 and the
4 heads 4*(c%4)..4*(c%4)+3.  Wq/Wk/Wv are column-sharded (each core gets the
256 rows of W* for its heads), Wo is row-sharded; the host sums the 4 partial
outputs per batch (the tensor-parallel all-reduce) and adds bo.

Device kernel (per core), matmuls in bf16 (f32r/f32 fallback via env):
  - qT/kT = W*_shard @ X.T   [256, 2048]  (head dim on partitions)
  - v     = X @ Wv_shard.T   [2048, 256]  (+ ones column per head for the
                                           softmax denominator)
  - per head: scoresT[s,t] = kT.T-tiles x qT  (causal tiles only),
    exp on ScalarE (scale=1/8), triangular-block mask on VectorE,
    outT[65, t] += v_ext.T @ attnT  (row 64 = denominator)
  - normalize per 512-token psum bank as soon as its accumulation stops
    (32-lane DVE reciprocal via stream transposes + gpsimd broadcast)
  - out_partial[t, :] = outTn.T-tiles @ WoT_shard, emitted per bank as soon
    as both head pairs are normalized (hides the store tail)

Schedule: q/k projections first (DMA-overlapped); the hp0/chunk0 score+exp
pipeline starts immediately after (ScalarE is the attention bottleneck, so
it must start early), with the v projection woven between score steps and
the attn@V matmuls deferred until v lands.  In later chunks the scores+exp
stream runs two key tiles ahead of the mask/attn@V/normalize consumers, and
each chunk emits the next chunk's first two score steps before its own AV
tail so the ScalarE exp queue never drains.
"""

import os
import sys

sys.path.insert(0, "/opt/trn_rl_repo")

import ml_dtypes
import numpy as np

import concourse.bass as bass  # noqa: F401
import concourse.bass2jax as bass2jax
import concourse.tile as tile
from concourse import bacc, mybir
from concourse.bass_utils import run_bass_kernel_spmd

# Optional NEFF tee for local profiling (active only when the env var is set).
_orig_rename = bass2jax.rename_neff_tensors_and_patch_header


def _tee_rename(neff_path, mapping):
    data = _orig_rename(neff_path, mapping)
    tee = os.environ.get("BASS_MHA_NEFF_TEE")
    if tee:
        try:
            with open(tee, "wb") as f:
                f.write(data)
        except OSError:
            pass
    return data


bass2jax.rename_neff_tensors_and_patch_header = _tee_rename

F32 = mybir.dt.float32
AF = mybir.ActivationFunctionType

S = 2048  # sequence length
D = 1024  # model dim
HL = 256  # local head width (4 heads x 64)
DK = 64  # head dim
N_SI = S // 128  # 16 token tiles (partition dim of scoresT)

MODE = os.environ.get("BASS_MHA_DTYPE", "bf16")  # bf16 | f32r | f32
if MODE == "bf16":
    DT = mybir.dt.bfloat16
    NPDT = ml_dtypes.bfloat16
elif MODE == "f32r":
    DT = mybir.dt.float32r
    NPDT = np.float32
else:
    DT = F32
    NPDT = np.float32

LAST_EXEC_NS = None
_CACHED_NC = None


def _prep(a):
    """Cast a host array to the kernel's compute dtype (with fp32r rounding
    matching the compiler's fp32_to_fp32r when in f32r mode)."""
    a = np.ascontiguousarray(np.asarray(a, np.float32))
    if MODE == "bf16":
        return a.astype(ml_dtypes.bfloat16)
    if MODE == "f32r":
        bits = a.view(np.uint32).astype(np.uint64)
        rounded = (bits + 0x7FF + ((bits >> 12) & 1)) & 0xFFFFF000
        return rounded.astype(np.uint32).view(np.float32).reshape(a.shape)
    return a


def _build_kernel(tc):
    nc = tc.nc
    qt = nc.dram_tensor("qt", [D, S], DT, kind="ExternalInput").ap()
    kt = nc.dram_tensor("kt", [D, S], DT, kind="ExternalInput").ap()
    vt = nc.dram_tensor("vt", [D, S], DT, kind="ExternalInput").ap()
    wqt = nc.dram_tensor("wqt", [D, HL], DT, kind="ExternalInput").ap()
    wkt = nc.dram_tensor("wkt", [D, HL], DT, kind="ExternalInput").ap()
    wvt = nc.dram_tensor("wvt", [D, HL], DT, kind="ExternalInput").ap()
    wot = nc.dram_tensor("wot", [HL, D], DT, kind="ExternalInput").ap()
    mtri = nc.dram_tensor("mtri", [128, 128], DT, kind="ExternalInput").ap()
    vones = nc.dram_tensor("vones", [128, N_SI, 4, 1], DT, kind="ExternalInput").ap()
    out = nc.dram_tensor("out", [S, D], F32, kind="ExternalOutput").ap()

    consts = tc.alloc_tile_pool(name="consts", bufs=1)
    persist = tc.alloc_tile_pool(name="persist", bufs=1)
    xt_pool = tc.alloc_tile_pool(name="xt", bufs=16)
    attn_pool = tc.alloc_tile_pool(name="attn", bufs=20)
    bc_pool = tc.alloc_tile_pool(name="bc", bufs=3)
    out_pool = tc.alloc_tile_pool(name="outsb", bufs=3)
    sc_psum = tc.alloc_tile_pool(name="scps", bufs=2, space="PSUM")
    ot_psum = tc.alloc_tile_pool(name="otps", bufs=1, space="PSUM")

    # --- constants ---
    wq_sb = consts.tile([128, 8, HL], DT, name="wq_sb")
    wk_sb = consts.tile([128, 8, HL], DT, name="wk_sb")
    wv_sb = consts.tile([128, 8, HL], DT, name="wv_sb")
    wo_sb = consts.tile([128, 2, D], DT, name="wo_sb")
    mtri_sb = consts.tile([128, 128], DT, name="mtri_sb")
    nc.sync.dma_start(out=wq_sb, in_=wqt.rearrange("(n p) c -> p n c", p=128))

    # --- persistent activations ---
    qT = [persist.tile([128, S], DT, name=f"qT{i}", tag=f"qT{i}") for i in range(2)]
    kT = [persist.tile([128, S], DT, name=f"kT{i}", tag=f"kT{i}") for i in range(2)]
    # v with an appended ones column per head: [token_tile, si, head, 65]
    v_sb = persist.tile([128, N_SI, 4, DK + 1], DT, name="v_sb", tag="v_sb")
    outTn = [
        persist.tile([128, S], DT, name=f"outTn{i}", tag=f"outTn{i}") for i in range(2)
    ]

    # --- PE warm-up during the initial DMA wait: junk matmuls keep the HAM
    # activity window busy so the first projection matmuls run fast.
    junk = consts.tile([128, 512], DT, name="junk")
    nc.vector.memset(junk, 0.0)
    wps = sc_psum.tile([128, 512], F32, name="warm", tag="sc")
    for _ in range(8):
        nc.tensor.matmul(wps, junk[:, 0:128], junk, start=True, stop=True)

    # --- input tile DMAs --------------------------------------------------
    def load_xts(src, prefix):
        tiles = []
        for d in range(8):
            xtile = xt_pool.tile([128, S], DT, name=f"{prefix}_{d}", tag="xt")
            nc.sync.dma_start(out=xtile, in_=src[128 * d : 128 * d + 128, :])
            tiles.append(xtile)
        return tiles

    # --- q/k projections: psum[dq_tile, t] += wT[dtile, dq_tile].T @ xT[dtile, t]
    # One [128, 1024] psum group per (hp, token-half); while the ot banks hold
    # no accumulators ("quad" mode) rotate through all four psum slots so
    # groups never stall; once attn@V accumulators go live, rotate sc only.
    PSUM_TAGS = ["sc", "sc", "ot0", "ot1"]
    psum_rot = [0]
    psum_mode = ["quad"]

    def next_psum_tile(name):
        if psum_mode[0] == "quad":
            tag = PSUM_TAGS[psum_rot[0] % 4]
            psum_rot[0] += 1
        else:
            tag = "sc"
        pool = sc_psum if tag == "sc" else ot_psum
        return pool.tile([128, 1024], F32, name=name, tag=tag)

    def proj_group(xts, w_sb, hp, half, dst):
        ps = next_psum_tile(f"pj_{hp}_{half}")
        for d in range(8):
            for j in range(2):
                js = slice(512 * j, 512 * j + 512)
                ja = slice(1024 * half + 512 * j, 1024 * half + 512 * j + 512)
                nc.tensor.matmul(
                    ps[:, js],
                    w_sb[:, d, 128 * hp : 128 * hp + 128],
                    xts[d][:, ja],
                    start=(d == 0),
                    stop=(d == 7),
                )
        cols = slice(1024 * half, 1024 * half + 1024)
        nc.vector.tensor_copy(out=dst[:, cols], in_=ps)

    qxs = load_xts(qt, "xq")
    nc.sync.dma_start(out=wk_sb, in_=wkt.rearrange("(n p) c -> p n c", p=128))
    for hp in range(2):
        for half in range(2):
            proj_group(qxs, wq_sb, hp, half, qT[hp])
    kxs = load_xts(kt, "xk")
    nc.sync.dma_start(out=wv_sb, in_=wvt.rearrange("(n p) c -> p n c", p=128))
    nc.sync.dma_start(out=mtri_sb, in_=mtri)
    nc.sync.dma_start(out=v_sb[:, :, :, DK : DK + 1], in_=vones)
    for half in range(2):
        proj_group(kxs, wk_sb, 0, half, kT[0])
    vxs = load_xts(vt, "xv")
    nc.sync.dma_start(out=wo_sb, in_=wot.rearrange("(n p) c -> p n c", p=128))

    # --- attention helpers ------------------------------------------------
    def score_step(hp, h, si, ch):
        """Scores + exp + mask for head h (0..3), key tile si, chunk ch.
        Returns the bf16 attention-weights tile."""
        ch_lo = 1024 * ch
        t_min = 128 * si
        hr = 64 * (h % 2)
        banks = [tj for tj in (2 * ch, 2 * ch + 1) if 512 * tj + 512 > t_min]
        sc = next_psum_tile(f"sc_{h}_{si}_{ch}")
        for tj in banks:
            a = max(512 * tj, t_min)  # skip sub-diagonal columns
            rel = slice(a - ch_lo, 512 * tj - ch_lo + 512)
            nc.tensor.matmul(
                sc[:, rel],
                kT[hp][hr : hr + 64, t_min : t_min + 128],
                qT[hp][hr : hr + 64, a : 512 * tj + 512],
                start=True,
                stop=True,
            )
        att = attn_pool.tile([128, 1024], DT, name=f"at_{h}_{si}_{ch}", tag="at")
        off = max(t_min - ch_lo, 0)
        nc.scalar.activation(att[:, off:1024], sc[:, off:1024], AF.Exp, scale=0.125)
        if ch_lo <= t_min < ch_lo + 1024:
            nc.vector.tensor_mul(
                att[:, off : off + 128], att[:, off : off + 128], mtri_sb
            )
        return att

    def av_step(ot, h, si, ch, att):
        """outT[65, t] += v_ext.T @ attnT for head h, key tile si."""
        ch_lo = 1024 * ch
        t_min = 128 * si
        for tj in (2 * ch, 2 * ch + 1):
            if 512 * tj + 512 <= t_min:
                continue
            a = max(512 * tj, t_min)
            b = 512 * tj + 512
            nc.tensor.matmul(
                ot[0:65, a - ch_lo : b - ch_lo],
                v_sb[:, si, h, :],
                att[:, a - ch_lo : b - ch_lo],
                start=(si == 0),
                stop=(si == 4 * tj + 3),
                skip_group_check=True,
            )

    def norm_bank(hp, h, ch, tj, ot):
        """Normalize one completed 512-token psum bank of head h: outTn
        = outT[0:64] * broadcast(1/denom).  denom is outT row 64.  The
        reciprocal runs on 32 DVE lanes via 32x32 stream transposes (a
        single-partition reciprocal is ~7ns/element — far too slow)."""
        hr = 64 * (h % 2)
        rel = slice(512 * (tj - 2 * ch), 512 * (tj - 2 * ch) + 512)
        dts = bc_pool.tile([32, 512], F32, name=f"dts_{h}_{ch}_{tj}", tag="dts")
        dtt = bc_pool.tile([32, 512], F32, name=f"dtt_{h}_{ch}_{tj}", tag="dtt")
        dtr = bc_pool.tile([32, 512], F32, name=f"dtr_{h}_{ch}_{tj}", tag="dtr")
        nc.vector.tensor_copy(out=dts[0:1, :], in_=ot[64:65, rel])
        nc.vector.transpose(dtt, dts)
        col0 = dtt.rearrange("p (b c) -> p b c", c=32)[:, :, 0]
        nc.vector.reciprocal(col0, col0)
        nc.vector.transpose(dtr, dtt)
        bcb = bc_pool.tile([64, 512], F32, name=f"bcb_{h}_{ch}_{tj}", tag="bcb")
        nc.gpsimd.partition_broadcast(bcb, dtr[0:1, :])
        nc.vector.tensor_mul(
            outTn[hp][hr : hr + 64, 512 * tj : 512 * tj + 512],
            ot[0:64, rel],
            bcb,
        )

    def outproj_tile(tt):
        """out[t, :] = sum_k outTn[k, tt].T @ woT[k, :] for one token tile."""
        ts = slice(128 * tt, 128 * tt + 128)
        ps = next_psum_tile(f"op_{tt}")
        for kk in range(2):
            for nj in range(2):
                js = slice(512 * nj, 512 * nj + 512)
                nc.tensor.matmul(
                    ps[:, js],
                    outTn[kk][:, ts],
                    wo_sb[:, kk, js],
                    start=(kk == 0),
                    stop=(kk == 1),
                )
        osb = out_pool.tile([128, D], F32, name=f"osb_{tt}", tag="osb")
        nc.vector.tensor_copy(out=osb, in_=ps)
        nc.sync.dma_start(out=out[ts, :], in_=osb)

    def v_group(g):
        """v projection for key tiles 4g..4g+3 in one [128, 1024] psum group."""
        ps = next_psum_tile(f"vps_{g}")
        for k in range(4):
            si = 4 * g + k
            for d in range(8):
                nc.tensor.matmul(
                    ps[:, 256 * k : 256 * k + 256],
                    vxs[d][:, 128 * si : 128 * si + 128],
                    wv_sb[:, d, :],
                    start=(d == 0),
                    stop=(d == 7),
                )
        nc.vector.tensor_copy(
            out=v_sb[:, 4 * g : 4 * g + 4, :, 0:DK],
            in_=ps.rearrange("p (s h d) -> p s h d", s=4, h=4),
        )

    # --- Phase B: hp0/ch0 scores+exp immediately (ScalarE must start early);
    # the k-hp1 projection and the v projection are woven between score steps
    # (matching their DMA arrival), and attn@V is deferred until v lands.
    atts0 = {}
    for si in range(8):
        for h in (0, 1):
            atts0[(h, si)] = score_step(0, h, si, 0)
        if si in (1, 2):
            proj_group(kxs, wk_sb, 1, si - 1, kT[1])
        elif si >= 4:
            v_group(si - 4)
    psum_mode[0] = "duo"
    # prefix scores of the next chunk keep ScalarE fed while the deferred
    # attn@V matmuls drain on the PE.
    pre = {
        (hl, si): score_step(0, hl, si, 1) for si in (0, 1) for hl in (0, 1)
    }
    ot0 = {
        h: ot_psum.tile([128, 1024], F32, name=f"outT_{h}_0", tag=f"ot{h}")
        for h in (0, 1)
    }
    pend_op = []

    def _triggers(hp, ch, avsi, ot):
        for tj in (2 * ch, 2 * ch + 1):
            if avsi == 4 * tj + 3:
                for h in (2 * hp, 2 * hp + 1):
                    norm_bank(hp, h, ch, tj, ot[h])
                if hp == 1:
                    pend_op.extend(range(4 * tj, 4 * tj + 4))

    for si in range(8):
        for h in (0, 1):
            av_step(ot0[h], h, si, 0, atts0[(h, si)])
        _triggers(0, 0, si, ot0)

    # --- remaining chunks: software-pipelined score/exp stream with the
    # attn@V + mask consumers trailing two key tiles behind, so the ScalarE
    # exp queue never waits on the mask/AV/normalize chain.  hp1 interleaves
    # the output projection for banks fully normalized in both head pairs.
    def run_chunk(hp, ch, pre, nxt):
        heads = (2 * hp, 2 * hp + 1)
        ot = {
            h: ot_psum.tile([128, 1024], F32, name=f"outT_{h}_{ch}", tag=f"ot{h % 2}")
            for h in heads
        }
        atts = dict(pre)
        si_max = 8 * ch + 7
        for si in range(2, si_max + 1):
            for h in heads:
                atts[(h % 2, si)] = score_step(hp, h, si, ch)
            avsi = si - 2
            for h in heads:
                av_step(ot[h], h, avsi, ch, atts.pop((h % 2, avsi)))
            _triggers(hp, ch, avsi, ot)
            if pend_op and si % 2 == 0:
                outproj_tile(pend_op.pop(0))
        npre = None
        if nxt is not None:
            nhp, nch = nxt
            npre = {
                (hl, si): score_step(nhp, 2 * nhp + hl, si, nch)
                for si in (0, 1)
                for hl in (0, 1)
            }
        for avsi in (si_max - 1, si_max):
            for h in heads:
                av_step(ot[h], h, avsi, ch, atts.pop((h % 2, avsi)))
            _triggers(hp, ch, avsi, ot)
        return npre

    pre = run_chunk(0, 1, pre, (1, 0))
    pre = run_chunk(1, 0, pre, (1, 1))
    run_chunk(1, 1, pre, None)
    for tt in pend_op:
        outproj_tile(tt)

    for pool in (
        ot_psum,
        sc_psum,
        out_pool,
        bc_pool,
        attn_pool,
        xt_pool,
        persist,
        consts,
    ):
        pool.release()


def _get_nc():
    global _CACHED_NC
    if _CACHED_NC is None:
        nc = bacc.Bacc("TRN2", target_bir_lowering=False, debug=False)
        with tile.TileContext(nc) as tc:
            _build_kernel(tc)
        nc.compile()
        _CACHED_NC = nc
    return _CACHED_NC


def kernel(Q, K, V, mask, Wq, Wk, Wv, Wo, bo):
    global LAST_EXEC_NS
    nc = _get_nc()
    mtri = np.triu(np.ones((128, 128), dtype=np.float32))
    in_maps = []
    for c in range(8):
        b, hg = c // 4, c % 4
        rs = slice(HL * hg, HL * hg + HL)
        in_maps.append(
            {
                "qt": _prep(np.asarray(Q, np.float32)[b].T),
                "kt": _prep(np.asarray(K, np.float32)[b].T),
                "vt": _prep(np.asarray(V, np.float32)[b].T),
                "wqt": _prep(np.asarray(Wq, np.float32)[rs].T),
                "wkt": _prep(np.asarray(Wk, np.float32)[rs].T),
                "wvt": _prep(np.asarray(Wv, np.float32)[rs].T),
                "wot": _prep(np.asarray(Wo, np.float32)[:, rs].T),
                "mtri": _prep(mtri),
                "vones": _prep(np.ones((128, N_SI, 4, 1), np.float32)),
            }
        )
    trace = os.environ.get("BASS_MHA_TRACE", "") == "1"
    res = run_bass_kernel_spmd(nc, in_maps, core_ids=list(range(8)), trace=trace)
    LAST_EXEC_NS = res.exec_time_ns
    outs = [res.results[c]["out"] for c in range(8)]
    bo = np.asarray(bo, np.float32)
    full = np.stack(
        [
            outs[0] + outs[1] + outs[2] + outs[3] + bo,
            outs[4] + outs[5] + outs[6] + outs[7] + bo,
        ]
    ).astype(np.float32)
    return full
